# revision 1
# baseline (speedup 1.0000x reference)
"""Cross-attention kernel for 8 Trainium2 NeuronCores.

Sharding: data-parallel over batch (B=2) x tensor-parallel over heads
(16 heads -> 4 groups of 4 heads).  Core c handles batch c//4, head
group c%4.  All operands are cast to bf16 on the host (halves the
input-DMA volume that gates attention start; rel-err budget is 2e-2).

Per core, for its 4 heads:
    Q^T = Wq_g^T x_b^T        [256, 2048]   (d-on-partitions layout)
    K^T = Wk_g^T y_b^T        [256, 2048]
    V   = y_b Wv_g            [2048, 256]   (n-on-partitions, built
                               just-in-time inside the first i-block)
    S^T_h = K_h Q_h^T / 8; P^T = exp(S^T)   (ScalarE, the bottleneck)
    O^T_h (+row sums via a ones-column in V) = [V_h|1]^T P^T
    partial = (O^T/rowsum)^T Wp_g           [2048, 1024]

Timeline: x streams first (Q), then y (K); V and the per-i-block
output projection are interleaved into the attention loop's PE slack
(ScalarE exp = ~132us is the phase bottleneck).  PSUM: 4 banks S
double-buffer + 2 banks O accumulators (two 65-partition chains packed
per bank via the start=True bank-wide has_written clear) + 2 banks
shared V/proj pool.
"""

import numpy as np

B = 2
N = 2048          # query sequence length
M = 2048          # key sequence length
DIM = 1024
HEAD_DIM = 64
SCALE = HEAD_DIM ** -0.5
NCORES = 8
GH = 4            # heads per core
J = GH * HEAD_DIM # 256 projected columns per core
KC = DIM // 128   # 8 contraction chunks
NT = M // 128     # 16 key tiles
IBS = 256         # i-block size
IB = N // IBS     # 8 i-blocks

_NC = None


def _build():
    from contextlib import ExitStack

    import concourse.bass as bass
    import concourse.tile as tile
    from concourse import bacc, mybir
    from concourse.bass import ts, ds

    f32 = mybir.dt.float32
    bf16 = mybir.dt.bfloat16
    Exp = mybir.ActivationFunctionType.Exp

    nc = bacc.Bacc("TRN2", target_bir_lowering=False, debug=False,
                   num_devices=NCORES)
    xT = nc.dram_tensor("xT", [DIM, N], bf16, kind="ExternalInput").ap()
    yT = nc.dram_tensor("yT", [DIM, M], bf16, kind="ExternalInput").ap()
    wq = nc.dram_tensor("wq", [DIM, J], bf16, kind="ExternalInput").ap()
    wk = nc.dram_tensor("wk", [DIM, J], bf16, kind="ExternalInput").ap()
    wv = nc.dram_tensor("wv", [DIM, J], bf16, kind="ExternalInput").ap()
    wp = nc.dram_tensor("wp", [J, DIM], bf16, kind="ExternalInput").ap()
    out = nc.dram_tensor("out", [N, DIM], bf16, kind="ExternalOutput").ap()

    with tile.TileContext(nc) as tc, ExitStack() as top:
        wpool = top.enter_context(tc.tile_pool(name="weights", bufs=1))
        wq_sb = wpool.tile([128, KC, J], bf16, name="wq_sb")
        wk_sb = wpool.tile([128, KC, J], bf16, name="wk_sb")
        wv_sb = wpool.tile([128, KC, J], bf16, name="wv_sb")
        wp_sb = wpool.tile([128, 2, DIM], bf16, name="wp_sb")
        scr = wpool.tile([1, 2], bf16, name="scr")

        big = top.enter_context(tc.tile_pool(name="big", bufs=1))
        xt = big.tile([128, KC, N], bf16, name="xt")
        yt = big.tile([128, KC, M], bf16, name="yt")
        QT = [big.tile([128, N], bf16, name=f"qt{t}") for t in range(2)]
        KT = [big.tile([128, M], bf16, name=f"kt{t}") for t in range(2)]
        V_sb = big.tile([128, NT, GH, HEAD_DIM + 1], bf16, name="v_sb")
        # ones column for the row-sum trick; V evacuation overwrites 0:64
        nc.vector.memset(V_sb, 1.0)
        # preload the Exp table so the first real activation doesn't pay
        # the ~2.7us ACT_TABLE_LOAD on the critical path
        nc.vector.memset(scr, 0.0)
        nc.scalar.activation(scr, scr, Exp, bias=0.0, scale=1.0)

        # ---- input streams: x first (Q is computed first), then y ----
        wq_r = wq.rearrange("(c p) j -> p c j", p=128)
        wk_r = wk.rearrange("(c p) j -> p c j", p=128)
        nc.sync.dma_start(wq_sb, wq_r)
        for c in range(KC):
            q = nc.sync if c % 2 == 0 else nc.scalar
            q.dma_start(xt[:, c, :], xT[ts(c, 128), :])
        nc.scalar.dma_start(wk_sb, wk_r)
        for c in range(KC):
            q = nc.sync if c % 2 == 0 else nc.scalar
            q.dma_start(yt[:, c, :], yT[ts(c, 128), :])
        nc.scalar.dma_start(wv_sb, wv.rearrange("(c p) j -> p c j", p=128))
        nc.scalar.dma_start(wp_sb, wp.rearrange("(t p) c -> p t c", p=128))

        # ---- Q^T then K^T (each: 8 psum banks, c-pipelined) ----------
        for name, wsb, src, dst in (("q", wq_sb, xt, QT), ("k", wk_sb, yt, KT)):
            with tc.tile_pool(name=f"{name}psum", bufs=1,
                              space="PSUM") as ppsum:
                ps = [ppsum.tile([128, 512], f32, name=f"{name}ps{t}")
                      for t in range(8)]
                for c in range(KC):
                    for jt in range(2):
                        for ic in range(4):
                            nc.tensor.matmul(
                                ps[jt * 4 + ic],
                                wsb[:, c, ts(jt, 128)],
                                src[:, c, ts(ic, 512)],
                                start=(c == 0), stop=(c == KC - 1))
                for jt in range(2):
                    for ic in range(4):
                        nc.vector.tensor_copy(dst[jt][:, ts(ic, 512)],
                                              ps[jt * 4 + ic])

        # ---- attention + JIT V + pipelined output projection ---------
        ppool = top.enter_context(tc.tile_pool(name="ppool", bufs=4))
        otpool = top.enter_context(tc.tile_pool(name="otpool", bufs=6))
        rpool = top.enter_context(tc.tile_pool(name="rpool", bufs=4))
        rbpool = top.enter_context(tc.tile_pool(name="rbpool", bufs=4))
        obpool = top.enter_context(tc.tile_pool(name="obpool", bufs=4))
        rdram = top.enter_context(tc.tile_pool(name="rdram", bufs=3,
                                               space="DRAM"))
        ot_tiles = {}

        def emit_v_tile(n, aux):
            vp = aux.tile([128, J], f32, name="vp", tag="aux")
            for c in range(KC):
                nc.tensor.matmul(
                    vp,
                    yt[:, c, ts(n, 128)],
                    wv_sb[:, c, :],
                    start=(c == 0), stop=(c == KC - 1))
            nc.vector.tensor_copy(
                V_sb[:, n, :, 0:HEAD_DIM],
                vp.rearrange("p (h d) -> p h d", h=GH))

        def emit_proj_piece(ib, piece, aux):
            icr, cc = divmod(piece, DIM // 512)
            op = aux.tile([128, 512], f32, name="op", tag="aux")
            for pr in range(2):
                nc.tensor.matmul(
                    op,
                    ot_tiles[(ib, pr)][:, ts(icr, 128)],
                    wp_sb[:, pr, ts(cc, 512)],
                    start=(pr == 0), stop=(pr == 1))
            ob = obpool.tile([128, 512], bf16, name="ob")
            nc.vector.tensor_copy(ob, op)
            nc.sync.dma_start(
                out[ds(ib * IBS + icr * 128, 128), ts(cc, 512)],
                ob)

        def emit_proj(ib, aux):
            for piece in range(4):
                emit_proj_piece(ib, piece, aux)

        with tc.tile_pool(name="spsum", bufs=2, space="PSUM") as spsum, \
             tc.tile_pool(name="opsum", bufs=1, space="PSUM") as opsum, \
             tc.tile_pool(name="auxp", bufs=2, space="PSUM") as auxp:
            emit_v_tile(0, auxp)
            emit_v_tile(1, auxp)
            for ib in range(IB):
                i_sl = ts(ib, IBS)
                oacc = [opsum.tile([HEAD_DIM + 1, 2, IBS], f32,
                                   name=f"oa{pr}") for pr in range(2)]
                pts = {}
                for n in range(NT):
                    if ib == 0 and n + 2 < NT:
                        emit_v_tile(n + 2, auxp)
                    if ib >= 2 and n in (3, 6, 9, 12):
                        emit_proj_piece(ib - 2, (n - 3) // 3, auxp)
                    # S^T for all 4 heads of this n into one psum tile;
                    # row-packed pairs run concurrently on the PE
                    sp = spsum.tile([128, 2, 2, IBS], f32, name="sp")
                    for pr in range(2):
                        nc.tensor.matmul(
                            sp[:, 0, pr, :],
                            KT[pr][0:64, ts(n, 128)],
                            QT[pr][0:64, i_sl],
                            start=True, stop=True, tile_position=(0, 0))
                        nc.tensor.matmul(
                            sp[:, 1, pr, :],
                            KT[pr][64:128, ts(n, 128)],
                            QT[pr][64:128, i_sl],
                            start=True, stop=True, tile_position=(64, 0))
                    pt = ppool.tile([128, 2, 2, IBS], bf16, name="pt")
                    nc.scalar.activation(pt, sp, Exp, bias=0.0,
                                         scale=float(SCALE))
                    pts[n] = pt
                    # O^T accumulation for the previous n (software
                    # pipeline keeps PE off the fresh exp's back)
                    if n > 0:
                        pt0 = pts.pop(n - 1)
                        for pr in range(2):
                            for lh in range(2):
                                nc.tensor.matmul(
                                    oacc[pr][:, lh, :],
                                    V_sb[:, n - 1, 2 * pr + lh, :],
                                    pt0[:, lh, pr, :],
                                    start=(n - 1 == 0 and lh == 0),
                                    stop=False)
                pt0 = pts.pop(NT - 1)
                for pr in range(2):
                    for lh in range(2):
                        nc.tensor.matmul(
                            oacc[pr][:, lh, :],
                            V_sb[:, NT - 1, 2 * pr + lh, :],
                            pt0[:, lh, pr, :],
                            start=False, stop=(lh == 1))
                # evacuate + normalize (reciprocal rowsum broadcast via
                # a DRAM round trip on the idle gpsimd DMA queue)
                blk = {}
                for pr in range(2):
                    ot = otpool.tile([128, IBS], bf16, name="ot")
                    nc.vector.tensor_copy(ot[0:64, :], oacc[pr][0:64, 0, :])
                    nc.vector.tensor_copy(ot[64:128, :], oacc[pr][0:64, 1, :])
                    rs_lo = rpool.tile([1, IBS], f32, name="rslo")
                    rs_hi = rpool.tile([1, IBS], f32, name="rshi")
                    nc.vector.tensor_copy(rs_lo, oacc[pr][64:65, 0, :])
                    nc.vector.tensor_copy(rs_hi, oacc[pr][64:65, 1, :])
                    blk[pr] = (ot, rs_lo, rs_hi)
                for pr in range(2):
                    ot, rs_lo, rs_hi = blk[pr]
                    rd = rdram.tile([2, IBS], f32, name="rd")
                    nc.sync.dma_start(rd[0:1, :], rs_lo)
                    nc.sync.dma_start(rd[1:2, :], rs_hi)
                    rb = rbpool.tile([128, IBS], f32, name="rb")
                    nc.sync.dma_start(rb[0:64, :],
                                        rd[0:1, :].partition_broadcast(64))
                    nc.sync.dma_start(rb[64:128, :],
                                        rd[1:2, :].partition_broadcast(64))
                    rb2 = rbpool.tile([128, IBS], f32, name="rb2")
                    nc.vector.reciprocal_approx_fast(rb2, rb)
                    nc.vector.tensor_mul(ot, ot, rb2)
                    ot_tiles[(ib, pr)] = ot
            emit_proj(IB - 2, auxp)
            emit_proj(IB - 1, auxp)

    nc.compile()
    return nc


def _get_nc():
    global _NC
    if _NC is None:
        _NC = _build()
    return _NC


def _shard_inputs(x, y, Wq, Wk, Wv, Wp):
    import ml_dtypes
    bf = ml_dtypes.bfloat16
    x = np.asarray(x, np.float32)
    y = np.asarray(y, np.float32)
    Wq = np.asarray(Wq, np.float32)
    Wk = np.asarray(Wk, np.float32)
    Wv = np.asarray(Wv, np.float32)
    Wp = np.asarray(Wp, np.float32)
    xT = [np.ascontiguousarray(x[b].T).astype(bf) for b in range(B)]
    yT = [np.ascontiguousarray(y[b].T).astype(bf) for b in range(B)]
    in_maps = []
    for c in range(NCORES):
        b, g = divmod(c, NCORES // B)
        sl = slice(g * J, (g + 1) * J)
        in_maps.append({
            "xT": xT[b],
            "yT": yT[b],
            "wq": np.ascontiguousarray(Wq[:, sl]).astype(bf),
            "wk": np.ascontiguousarray(Wk[:, sl]).astype(bf),
            "wv": np.ascontiguousarray(Wv[:, sl]).astype(bf),
            "wp": np.ascontiguousarray(Wp[sl, :]).astype(bf),
        })
    return in_maps


def run(inputs, trace=False, **spmd_kwargs):
    from concourse.bass_utils import run_bass_kernel_spmd
    nc = _get_nc()
    in_maps = _shard_inputs(inputs["x"], inputs["y"], inputs["Wq"],
                            inputs["Wk"], inputs["Wv"], inputs["Wp"])
    res = run_bass_kernel_spmd(nc, in_maps, core_ids=list(range(NCORES)),
                               trace=trace, **spmd_kwargs)
    bp = np.asarray(inputs["bp"], np.float32)
    gpb = NCORES // B
    full = np.empty((B, N, DIM), np.float32)
    for b in range(B):
        acc = res.results[b * gpb]["out"].astype(np.float32)
        for g in range(1, gpb):
            acc = acc + res.results[b * gpb + g]["out"].astype(np.float32)
        full[b] = acc + bp
    return full, res


def kernel(**inputs):
    out, _ = run(inputs, trace=False)
    return out



# revision 2
# speedup vs baseline: 1.0506x; 1.0506x over previous
"""Cross-attention kernel for 8 Trainium2 NeuronCores.

Sharding: data-parallel over batch (B=2) x tensor-parallel over heads
(16 heads -> 4 groups of 4 heads).  Core c handles batch c//4, head
group c%4.  All operands are cast to bf16 on the host (halves the
input-DMA volume that gates attention start; rel-err budget is 2e-2).

ACT (ScalarE exp, ~147us busy) is the hard roofline: every other
engine's work is scheduled into its shadow.

Per core, for its 4 heads:
    K^T = Wk_g^T y_b^T        [256, 2048]   (y streams first; K GEMM
                               c-pipelined with the y chunk DMAs)
    Q^T_ib = Wq_g^T x_ib^T    [256, 256]    (JIT per i-block)
    V   = y_b Wv_g            [2048, 256]   (JIT inside i-block 0)
    S^T_h = K_h Q_h^T / 8; P^T = exp(S^T)   (ScalarE, the bottleneck)
    O^T_h (+row sums via a ones-column in V) = [V_h|1]^T P^T
    partial = (O^T/rowsum)^T Wp_g           [2048, 1024]

Pipeline: chunk k = (ib, n); tensor-queue emission leads ACT by one
slot (S(k+1) emitted before AV(k-1)) so i-block boundaries never
stall the ACT queue; proj lags one i-block.  PSUM: 4 banks S
double-buffer + 2 banks O accumulators + 2 banks shared V/Q/proj
pool.
"""

import numpy as np

B = 2
N = 2048          # query sequence length
M = 2048          # key sequence length
DIM = 1024
HEAD_DIM = 64
SCALE = HEAD_DIM ** -0.5
NCORES = 8
GH = 4            # heads per core
J = GH * HEAD_DIM # 256 projected columns per core
KC = DIM // 128   # 8 contraction chunks
NT = M // 128     # 16 key tiles
IBS = 256         # i-block size
IB = N // IBS     # 8 i-blocks
TOT = IB * NT     # 128 chunks

_NC = None


def _build():
    from contextlib import ExitStack

    import concourse.bass as bass
    import concourse.tile as tile
    from concourse import bacc, mybir
    from concourse.bass import ts, ds

    f32 = mybir.dt.float32
    bf16 = mybir.dt.bfloat16
    Exp = mybir.ActivationFunctionType.Exp

    nc = bacc.Bacc("TRN2", target_bir_lowering=False, debug=False,
                   num_devices=NCORES)
    xT = nc.dram_tensor("xT", [DIM, N], bf16, kind="ExternalInput").ap()
    yT = nc.dram_tensor("yT", [DIM, M], bf16, kind="ExternalInput").ap()
    wq = nc.dram_tensor("wq", [DIM, J], bf16, kind="ExternalInput").ap()
    wk = nc.dram_tensor("wk", [DIM, J], bf16, kind="ExternalInput").ap()
    wv = nc.dram_tensor("wv", [DIM, J], bf16, kind="ExternalInput").ap()
    wp = nc.dram_tensor("wp", [J, DIM], bf16, kind="ExternalInput").ap()
    out = nc.dram_tensor("out", [N, DIM], bf16, kind="ExternalOutput").ap()

    with tile.TileContext(nc) as tc, ExitStack() as top:
        wpool = top.enter_context(tc.tile_pool(name="weights", bufs=1))
        wq_sb = wpool.tile([128, KC, J], bf16, name="wq_sb")
        wk_sb = wpool.tile([128, KC, J], bf16, name="wk_sb")
        wv_sb = wpool.tile([128, KC, J], bf16, name="wv_sb")
        wp_sb = wpool.tile([128, 2, DIM], bf16, name="wp_sb")
        scr = wpool.tile([1, 2], bf16, name="scr")

        big = top.enter_context(tc.tile_pool(name="big", bufs=1))
        xt = big.tile([128, KC, N], bf16, name="xt")
        yt = big.tile([128, KC, M], bf16, name="yt")
        KT = [big.tile([128, M], bf16, name=f"kt{t}") for t in range(2)]
        V_sb = big.tile([128, NT, GH, HEAD_DIM + 1], bf16, name="v_sb")
        qpool = top.enter_context(tc.tile_pool(name="qpool", bufs=2))
        # ones column for the row-sum trick; V evacuation overwrites 0:64
        nc.vector.memset(V_sb, 1.0)
        # preload the Exp table so the first real activation doesn't pay
        # the ~2.7us ACT_TABLE_LOAD on the critical path
        nc.vector.memset(scr, 0.0)
        nc.scalar.activation(scr, scr, Exp, bias=0.0, scale=1.0)

        # ---- input streams: y first (K gates the first exp), then x --
        nc.sync.dma_start(wk_sb, wk.rearrange("(c p) j -> p c j", p=128))
        nc.scalar.dma_start(wq_sb, wq.rearrange("(c p) j -> p c j", p=128))
        for c in range(KC):
            q = nc.sync if c % 2 == 0 else nc.scalar
            q.dma_start(yt[:, c, :], yT[ts(c, 128), :])
        nc.sync.dma_start(wv_sb, wv.rearrange("(c p) j -> p c j", p=128))
        for c in range(KC):
            q = nc.scalar if c % 2 == 0 else nc.sync
            q.dma_start(xt[:, c, 0:IBS], xT[ts(c, 128), 0:IBS])
        nc.scalar.dma_start(wp_sb, wp.rearrange("(t p) c -> p t c", p=128))
        for c in range(KC):
            q = nc.sync if c % 2 == 0 else nc.scalar
            q.dma_start(xt[:, c, IBS:DIM], xT[ts(c, 128), IBS:DIM])
        for c in range(KC):
            q = nc.scalar if c % 2 == 0 else nc.sync
            q.dma_start(xt[:, c, DIM:N], xT[ts(c, 128), DIM:N])

        # ---- K^T (8 psum banks, c-pipelined with the y DMA chunks) ---
        with tc.tile_pool(name="kpsum", bufs=1, space="PSUM") as kpsum:
            kps = [kpsum.tile([128, 512], f32, name=f"kps{t}")
                   for t in range(8)]
            for c in range(KC):
                for jt in range(2):
                    for ic in range(4):
                        nc.tensor.matmul(
                            kps[jt * 4 + ic],
                            wk_sb[:, c, ts(jt, 128)],
                            yt[:, c, ts(ic, 512)],
                            start=(c == 0), stop=(c == KC - 1))
            # evacuate ic-ascending so S(n=0..3) only waits the first two
            for ic in range(4):
                for jt in range(2):
                    nc.vector.tensor_copy(KT[jt][:, ts(ic, 512)],
                                          kps[jt * 4 + ic])

        # ---- attention: chunk pipeline over (ib, n) ------------------
        ppool = top.enter_context(tc.tile_pool(name="ppool", bufs=4))
        otpool = top.enter_context(tc.tile_pool(name="otpool", bufs=6))
        rpool = top.enter_context(tc.tile_pool(name="rpool", bufs=4))
        rbpool = top.enter_context(tc.tile_pool(name="rbpool", bufs=4))
        obpool = top.enter_context(tc.tile_pool(name="obpool", bufs=4))
        rdram = top.enter_context(tc.tile_pool(name="rdram", bufs=3,
                                               space="DRAM"))
        ot_tiles = {}
        q_tiles = {}
        PIECE_SLOT = {4: 0, 7: 1, 10: 2, 13: 3}

        with tc.tile_pool(name="spsum", bufs=2, space="PSUM") as spsum, \
             tc.tile_pool(name="opsum", bufs=1, space="PSUM") as opsum, \
             tc.tile_pool(name="auxp", bufs=2, space="PSUM") as auxp:

            def emit_q_half(ib, jt):
                if jt == 0:
                    q_tiles[ib] = qpool.tile([128, 2, IBS], bf16,
                                             name="qt")
                qp = auxp.tile([128, IBS], f32, name="qp", tag="aux")
                i_sl = ts(ib, IBS)
                for c in range(KC):
                    nc.tensor.matmul(
                        qp,
                        wq_sb[:, c, ts(jt, 128)],
                        xt[:, c, i_sl],
                        start=(c == 0), stop=(c == KC - 1))
                nc.vector.tensor_copy(q_tiles[ib][:, jt, :], qp)

            def emit_v_tile(n):
                vp = auxp.tile([128, J], f32, name="vp", tag="aux")
                for c in range(KC):
                    nc.tensor.matmul(
                        vp,
                        yt[:, c, ts(n, 128)],
                        wv_sb[:, c, :],
                        start=(c == 0), stop=(c == KC - 1))
                nc.vector.tensor_copy(
                    V_sb[:, n, :, 0:HEAD_DIM],
                    vp.rearrange("p (h d) -> p h d", h=GH))

            def emit_proj_piece(ib, piece):
                icr, cc = divmod(piece, DIM // 512)
                op = auxp.tile([128, 512], f32, name="op", tag="aux")
                for pr in range(2):
                    nc.tensor.matmul(
                        op,
                        ot_tiles[(ib, pr)][:, ts(icr, 128)],
                        wp_sb[:, pr, ts(cc, 512)],
                        start=(pr == 0), stop=(pr == 1))
                ob = obpool.tile([128, 512], bf16, name="ob")
                nc.vector.tensor_copy(ob, op)
                nc.sync.dma_start(
                    out[ds(ib * IBS + icr * 128, 128), ts(cc, 512)],
                    ob)

            sps = {}
            pts = {}
            oaccs = {}

            def emit_s(k):
                ib, n = divmod(k, NT)
                qt = q_tiles[ib]
                sp = spsum.tile([128, 2, 2, IBS], f32, name="sp")
                for pr in range(2):
                    nc.tensor.matmul(
                        sp[:, 0, pr, :],
                        KT[pr][0:64, ts(n, 128)],
                        qt[0:64, pr, :],
                        start=True, stop=True, tile_position=(0, 0))
                    nc.tensor.matmul(
                        sp[:, 1, pr, :],
                        KT[pr][64:128, ts(n, 128)],
                        qt[64:128, pr, :],
                        start=True, stop=True, tile_position=(64, 0))
                sps[k] = sp

            def emit_exp(k):
                pt = ppool.tile([128, 2, 2, IBS], bf16, name="pt")
                nc.scalar.activation(pt, sps.pop(k), Exp, bias=0.0,
                                     scale=float(SCALE))
                pts[k] = pt

            def emit_av(k):
                ib, n = divmod(k, NT)
                if n == 0:
                    oaccs[ib] = [opsum.tile([HEAD_DIM + 1, 2, IBS], f32,
                                            name=f"oa{pr}")
                                 for pr in range(2)]
                oacc = oaccs[ib]
                pt = pts.pop(k)
                for pr in range(2):
                    for lh in range(2):
                        nc.tensor.matmul(
                            oacc[pr][:, lh, :],
                            V_sb[:, n, 2 * pr + lh, :],
                            pt[:, lh, pr, :],
                            start=(n == 0 and lh == 0),
                            stop=(n == NT - 1 and lh == 1))
                if n == NT - 1:
                    emit_norm(ib)

            def emit_norm(ib):
                # evacuate + normalize (reciprocal rowsum broadcast via
                # a DRAM round trip)
                oacc = oaccs.pop(ib)
                blk = {}
                for pr in range(2):
                    ot = otpool.tile([128, IBS], bf16, name="ot")
                    nc.vector.tensor_copy(ot[0:64, :], oacc[pr][0:64, 0, :])
                    nc.vector.tensor_copy(ot[64:128, :],
                                          oacc[pr][0:64, 1, :])
                    rs_lo = rpool.tile([1, IBS], f32, name="rslo")
                    rs_hi = rpool.tile([1, IBS], f32, name="rshi")
                    nc.vector.tensor_copy(rs_lo, oacc[pr][64:65, 0, :])
                    nc.vector.tensor_copy(rs_hi, oacc[pr][64:65, 1, :])
                    blk[pr] = (ot, rs_lo, rs_hi)
                for pr in range(2):
                    ot, rs_lo, rs_hi = blk[pr]
                    rd = rdram.tile([2, IBS], f32, name="rd")
                    nc.sync.dma_start(rd[0:1, :], rs_lo)
                    nc.sync.dma_start(rd[1:2, :], rs_hi)
                    rb = rbpool.tile([128, IBS], f32, name="rb")
                    nc.sync.dma_start(rb[0:64, :],
                                      rd[0:1, :].partition_broadcast(64))
                    nc.sync.dma_start(rb[64:128, :],
                                      rd[1:2, :].partition_broadcast(64))
                    rb2 = rbpool.tile([128, IBS], f32, name="rb2")
                    nc.vector.reciprocal_approx_fast(rb2, rb)
                    nc.vector.tensor_mul(ot, ot, rb2)
                    ot_tiles[(ib, pr)] = ot

            emit_q_half(0, 0)
            emit_q_half(0, 1)
            emit_v_tile(0)
            for k in range(TOT + 2):
                if k < TOT:
                    emit_s(k)
                    ib, n = divmod(k, NT)
                    if ib == 0 and n + 1 < NT:
                        emit_v_tile(n + 1)
                    if n == 8 and ib + 1 < IB:
                        emit_q_half(ib + 1, 0)
                    if n == 10 and ib + 1 < IB:
                        emit_q_half(ib + 1, 1)
                    if ib >= 1 and n in PIECE_SLOT:
                        emit_proj_piece(ib - 1, PIECE_SLOT[n])
                    emit_exp(k)
                if k >= 2:
                    emit_av(k - 2)
            for piece in range(4):
                emit_proj_piece(IB - 1, piece)

    nc.compile()
    return nc


def _get_nc():
    global _NC
    if _NC is None:
        _NC = _build()
    return _NC


def _shard_inputs(x, y, Wq, Wk, Wv, Wp):
    import ml_dtypes
    bf = ml_dtypes.bfloat16
    x = np.asarray(x, np.float32)
    y = np.asarray(y, np.float32)
    Wq = np.asarray(Wq, np.float32)
    Wk = np.asarray(Wk, np.float32)
    Wv = np.asarray(Wv, np.float32)
    Wp = np.asarray(Wp, np.float32)
    xT = [np.ascontiguousarray(x[b].T).astype(bf) for b in range(B)]
    yT = [np.ascontiguousarray(y[b].T).astype(bf) for b in range(B)]
    in_maps = []
    for c in range(NCORES):
        b, g = divmod(c, NCORES // B)
        sl = slice(g * J, (g + 1) * J)
        in_maps.append({
            "xT": xT[b],
            "yT": yT[b],
            "wq": np.ascontiguousarray(Wq[:, sl]).astype(bf),
            "wk": np.ascontiguousarray(Wk[:, sl]).astype(bf),
            "wv": np.ascontiguousarray(Wv[:, sl]).astype(bf),
            "wp": np.ascontiguousarray(Wp[sl, :]).astype(bf),
        })
    return in_maps


def run(inputs, trace=False, **spmd_kwargs):
    from concourse.bass_utils import run_bass_kernel_spmd
    nc = _get_nc()
    in_maps = _shard_inputs(inputs["x"], inputs["y"], inputs["Wq"],
                            inputs["Wk"], inputs["Wv"], inputs["Wp"])
    res = run_bass_kernel_spmd(nc, in_maps, core_ids=list(range(NCORES)),
                               trace=trace, **spmd_kwargs)
    bp = np.asarray(inputs["bp"], np.float32)
    gpb = NCORES // B
    full = np.empty((B, N, DIM), np.float32)
    for b in range(B):
        acc = res.results[b * gpb]["out"].astype(np.float32)
        for g in range(1, gpb):
            acc = acc + res.results[b * gpb + g]["out"].astype(np.float32)
        full[b] = acc + bp
    return full, res


def kernel(**inputs):
    out, _ = run(inputs, trace=False)
    return out
